# revision 51
# baseline (speedup 1.0000x reference)
"""DynamicConv1d Trainium2 kernel.

Reference computation (per sample b):
    pooled = mean_L(x[b])                                 # [C_in]
    att    = softmax((relu(pooled @ W1.T) @ W2.T) / T)    # [K]
    agg_w  = sum_k att[k] * weight[k]                     # [C_out, C_in, KS]
    agg_b  = sum_k att[k] * bias[k]                       # [C_out]
    out[b] = conv1d(x[b], agg_w, pad=3) + agg_b[:, None]  # [C_out, L]

Sharding: data-parallel over batch, 8 samples per core on 8 cores.

Kernel strategy per core (8 samples):
  - Host pre-packs x into a "doubled" bf16 tensor xd [S, 128, L+6]:
    rows 0..63  = x zero-padded by 3 on each side,
    rows 64..127 = the same, shifted left by one element.
    A conv tap-pair (f, f+1) is then ONE K=128 matmul against a 512-wide
    window of xd; taps (0,1),(2,3),(4,5) use all 128 partitions and tap 6
    uses rows 0..63 only.  7 taps -> 4 matmuls per 512-wide output tile.
  - Host pre-packs weight banks into stationary lhsT layout
    wbk [K, 128, 4*128]: wbk[k, (f%2)*64+i, (f//2)*128+o] = weight[k,o,i,f].
  - pooled: ONE stride-2 DVE reduce over all 128 partitions (HW-measured:
    DVE reduce is charged per element read, so this halves its cost):
    even columns of the lower half sum even-indexed xp, even columns of
    the shifted upper half sum odd-indexed xp; the cross-partition
    recombine is free inside the attention matmul via duplicated W1
    (w1d [128, H], pre-scaled by 1/L).
  - attention: tiny fp32 matmuls; exp(logits/T) unnormalized on ACT with
    its sum via accum_out (logits/T is O(0.01) here, so skipping the
    softmax max-subtraction is safe); [e|sum] broadcast to all 128
    partitions with a ones[1,128] outer-product matmul, then copied once
    to SBUF so the psum slot frees; 1/sum is folded into the drain scale.
  - weight aggregation: bf16 tensor_scalar x4 (4x DVE mode, HW-verified)
    + tensor_tensor add tree (2x) -> per-sample bf16 lhsT; bias via an
    accum_out dot against the host-transposed bias [C_out, K].
  - conv: per sample, per group of 5 L-tiles: 4 matmuls into psum banks;
    ACT drains psum -> bf16 out staging applying scale=1/sum and the
    per-sample bias; chunk DMAs (on the second HWDGE ring) stream the
    staging rows to DRAM; host upcasts bf16 -> f32.
  - emission is software-pipelined `la` samples ahead (attention emitted
    at high scheduler priority) so the PE conv stream never waits on the
    attention tail; HW-measured cross-engine latencies (~1us/hop) make
    the deeper lookahead matter.
"""

from contextlib import ExitStack

import ml_dtypes
import numpy as np

import concourse.bass as bass
import concourse.mybir as mybir
from concourse import bacc
from concourse.bass_utils import run_bass_kernel_spmd
from concourse.tile import TileContext

# Problem constants (nn_DynamicConv1d, hardcoded per the grading contract).
BS, C_IN, L = 64, 64, 4096
C_OUT, KS, K = 128, 7, 4
HIDDEN = C_IN // 4
PAD, TEMP = 3, 30.0
N_CORES = 8
S = BS // N_CORES  # samples per core

F32 = mybir.dt.float32
BF16 = mybir.dt.bfloat16
AF = mybir.ActivationFunctionType
ALU = mybir.AluOpType

_NC_CACHE = {}


def build_nc(s=S, length=L, tile_n=512, conv_bufs=6, iters=1, out_bf16=1, loop_n=1,
             abl=0, la=3, group_n=3, la_att=2):
    # abl (ablation for timing): 1=no out-DMA, 2=also no drains, 3=also no
    # conv matmuls (loads+attention only), 4=x loads only
    """Build the single-core Bass program (same program runs SPMD on 8 cores)."""
    lp = length + 2 * PAD  # padded row length (4102)
    n_tiles = length // tile_n
    out_dt = BF16 if out_bf16 else F32

    nc = bacc.Bacc("TRN2")
    xd = nc.dram_tensor("xd", [s, 128, lp], BF16, kind="ExternalInput")
    w1d = nc.dram_tensor("w1d", [128, HIDDEN], F32, kind="ExternalInput")
    w2t = nc.dram_tensor("w2t", [HIDDEN, K], F32, kind="ExternalInput")
    wbk = nc.dram_tensor("wbk", [K, 128, 512], BF16, kind="ExternalInput")
    bkbt = nc.dram_tensor("bkbt", [C_OUT, K], F32, kind="ExternalInput")
    out = nc.dram_tensor("out", [s, C_OUT, length], out_dt, kind="ExternalOutput")

    with TileContext(nc) as tc, ExitStack() as ctx:
        singles = ctx.enter_context(tc.tile_pool(name="singles", bufs=1))
        xpool = ctx.enter_context(tc.tile_pool(name="xpool", bufs=1))
        waggp = ctx.enter_context(tc.tile_pool(name="waggp", bufs=1))
        aggtmp = ctx.enter_context(tc.tile_pool(name="aggtmp", bufs=2))
        outp = ctx.enter_context(tc.tile_pool(name="outp", bufs=3))
        smallw = ctx.enter_context(tc.tile_pool(name="smallw", bufs=4))
        psum_small = ctx.enter_context(
            tc.tile_pool(name="psum_small", bufs=8 - conv_bufs, space="PSUM")
        )
        psum_conv = ctx.enter_context(
            tc.tile_pool(name="psum_conv", bufs=conv_bufs, space="PSUM")
        )

        half = lp // 2

        def load_x(si):
            # two column-half DMAs so the pooled reduce can start on the
            # first half while the second streams in
            x_t = xpool.tile([128, lp], BF16, name=f"x_{si}")
            nc.sync.dma_start(out=x_t[:, 0:half], in_=xd.ap()[si][:, 0:half])
            nc.sync.dma_start(out=x_t[:, half:lp], in_=xd.ap()[si][:, half:lp])
            return x_t

        # Sample 0's x first: it heads the critical path.
        xs = [load_x(0)]

        # Replicated parameters, loaded once.
        w1d_sb = singles.tile([128, HIDDEN], F32)
        nc.sync.dma_start(out=w1d_sb, in_=w1d.ap())
        w2t_sb = singles.tile([HIDDEN, K], F32)
        nc.sync.dma_start(out=w2t_sb, in_=w2t.ap())
        bkbt_sb = singles.tile([C_OUT, K], F32)
        nc.sync.dma_start(out=bkbt_sb, in_=bkbt.ap())
        # All 4 weight banks side by side: column k*512 + c (bf16).
        wbk_sb = singles.tile([128, K * 512], BF16)
        for k in range(K):
            nc.sync.dma_start(
                out=wbk_sb[:, k * 512 : (k + 1) * 512], in_=wbk.ap()[k]
            )
        ones_sb = singles.tile([1, 128], F32)
        nc.vector.memset(ones_sb, 1.0)

        pooled = singles.tile([128, s], F32)
        att_bcast = singles.tile([128, K * s], F32)
        agg_bias = singles.tile([C_OUT, s], F32)

        for it in range(iters):
            wagg = [None] * s
            rse128 = [None] * s
            bias_n = [None] * s

            def pooled_part(si):
                # pooled sums: both partition halves carry the same x (the
                # upper is just shifted, pads are zero), so each full-row sum
                # equals the pooled sum; the matmul contraction over all 128
                # partitions adds them and W1 is pre-divided by 2.  The sum
                # itself rides as accum_out on a 4x-mode bf16 tensor_scalar
                # copy into a junk tile (TensorReduce has no DVE fast mode).
                x_even = xs[si].rearrange("p (c two) -> p c two", two=2)[:, :, 0]
                nc.vector.reduce_sum(
                    out=pooled[:, si : si + 1],
                    in_=x_even,
                    axis=mybir.AxisListType.X,
                )

            def att_part(si):
                # h = relu(W1 @ pooled); W1 duplicated so the 128-partition
                # contraction recombines the two half-sums.
                h_ps = psum_small.tile([HIDDEN, 1], F32, tag="ps_small", name="h_ps")
                nc.tensor.matmul(
                    h_ps, w1d_sb, pooled[:, si : si + 1], start=True, stop=True
                )
                h_sb = smallw.tile([HIDDEN, 1], F32, tag="h_sb", name="h_sb")
                nc.scalar.activation(h_sb, h_ps, AF.Relu)
                # logits (transposed): [1, K]
                lg_ps = psum_small.tile([1, K], F32, tag="ps_small", name="lg_ps")
                nc.tensor.matmul(lg_ps, h_sb, w2t_sb, start=True, stop=True)
                # e = exp(logits/TEMP) unnormalized (logits/TEMP is O(0.01)
                # here, so no max-subtraction is needed); e5 = [e_0..e_3, sum]
                e5 = smallw.tile([1, K + 1], F32, tag="e5", name="e5")
                nc.scalar.activation(
                    e5[:, 0:K],
                    lg_ps,
                    AF.Exp,
                    scale=1.0 / TEMP,
                    accum_out=e5[:, K : K + 1],
                )
                # broadcast [e | sum] over all 128 partitions in one outer
                # product; normalization is folded into the psum drain scale.
                ab_ps = psum_small.tile([128, K + 1], F32, tag="ps_small", name="ab_ps")
                nc.tensor.matmul(ab_ps, ones_sb, e5, start=True, stop=True)
                # single psum reader: copy to SBUF so the psum slot frees
                # immediately instead of waiting for all 6 agg consumers
                attb = smallw.tile([128, K + 1], F32, tag="attb", name="attb")
                nc.vector.tensor_copy(attb, ab_ps)
                rse_s = smallw.tile([128, 1], F32, tag="rse", name="rse")
                nc.vector.reciprocal(rse_s, attb[:, K : K + 1])
                rse128[si] = rse_s
                # unnormalized agg bias, then pre-scale by 1/sum for the drain
                junk = smallw.tile([C_OUT, K], F32, tag="junk", name="junk")
                nc.vector.scalar_tensor_tensor(
                    out=junk,
                    in0=bkbt_sb,
                    scalar=1.0,
                    in1=attb[:, 0:K],
                    op0=ALU.mult,
                    op1=ALU.mult,
                    accum_out=agg_bias[:, si : si + 1],
                )
                bn_s = smallw.tile([C_OUT, 1], F32, tag="bn", name="bn")
                nc.vector.tensor_tensor(
                    bn_s, agg_bias[:, si : si + 1], rse_s, ALU.mult
                )
                bias_n[si] = bn_s
                # aggregate the 4 weight banks -> per-sample bf16 lhsT.
                # All-bf16 tensor_scalar (4x DVE mode) + tensor_tensor adds
                # (2x); scalars read straight from psum (mode-exempt).
                m = []
                for k in range(K):
                    mk = aggtmp.tile([128, 512], BF16, tag=f"m{k}", name=f"m{k}")
                    nc.vector.tensor_scalar(
                        out=mk,
                        in0=wbk_sb[:, k * 512 : (k + 1) * 512],
                        scalar1=attb[:, k : k + 1],
                        scalar2=None,
                        op0=ALU.mult,
                    )
                    m.append(mk)
                a01 = aggtmp.tile([128, 512], BF16, tag="a01", name="a01")
                nc.vector.tensor_tensor(a01, m[0], m[1], ALU.add)
                a23 = aggtmp.tile([128, 512], BF16, tag="a23", name="a23")
                nc.vector.tensor_tensor(a23, m[2], m[3], ALU.add)
                wagg_s = waggp.tile([128, 512], BF16, name=f"wagg_{si}")
                nc.vector.tensor_tensor(wagg_s, a01, a23, ALU.add)
                wagg[si] = wagg_s

            def convs(si):
                if abl >= 3:
                    return
                o_sb = outp.tile([C_OUT, length], out_dt, tag="o_sb", name="o_sb")
                drained = 0
                for g0 in range(0, n_tiles, group_n):
                    gts = range(g0, min(g0 + group_n, n_tiles))
                    psums = [
                        psum_conv.tile(
                            [C_OUT, tile_n], F32, tag="conv", name="conv_ps"
                        )
                        for _ in gts
                    ]
                    for p in range(4):
                        if p < 3:
                            lhsT = wagg[si][:, p * 128 : (p + 1) * 128]
                        else:
                            lhsT = wagg[si][0:C_IN, 3 * 128 : 4 * 128]
                        off = 2 * p if p < 3 else 6
                        for ti, t in enumerate(gts):
                            col = t * tile_n + off
                            if p < 3:
                                rhs = xs[si][:, col : col + tile_n]
                            else:
                                rhs = xs[si][0:C_IN, col : col + tile_n]
                            nc.tensor.matmul(
                                psums[ti], lhsT, rhs, start=(p == 0), stop=(p == 3)
                            )
                    for ti, t in enumerate(gts):
                        if abl >= 2:
                            break
                        dst = o_sb[:, t * tile_n : (t + 1) * tile_n]
                        nc.scalar.activation(
                            dst,
                            psums[ti],
                            AF.Identity,
                            bias=bias_n[si],
                            scale=rse128[si],
                        )
                    # write out each drained chunk as soon as it's ready;
                    # the last sample's final group goes per-tile so the
                    # kernel tail isn't gated on one big DMA
                    if abl < 1:
                        if si == s - 1 and gts[-1] + 1 == n_tiles:
                            step = 1
                        else:
                            step = len(gts)
                        end = gts[-1] + 1
                        while drained < end:
                            d1 = min(drained + step, end)
                            d0c, d1c = drained * tile_n, d1 * tile_n
                            nc.scalar.dma_start(
                                out=out.ap()[si][:, d0c:d1c], in_=o_sb[:, d0c:d1c]
                            )
                            drained = d1

            # software pipeline: attention one sample ahead of convs
            def body():
                for si in range(len(xs), s):
                    xs.append(load_x(si))
                if abl >= 4:
                    return
                # 2-stage software pipeline in plain emission order: the
                # pooled reduce is prefetched `la` samples ahead (it gates
                # the whole attention chain on DVE), the rest of attention
                # one sample ahead, so the PE stream interleaves
                # [att-mms(s+1) | convs(s)] with all inputs already ready.
                # prologue interleaved: att_part(j) right after its own
                # reduce, so sample 0's agg chain is not queued behind the
                # DMA-gated lookahead reduces on the in-order DVE stream
                for j in range(min(la, s)):
                    pooled_part(j)
                    if j < min(la_att, s):
                        att_part(j)
                for si in range(s):
                    # att_part first: the DVE stream is in-order, and the
                    # lookahead reduce waits on its x DMA — emitting it
                    # before agg would block ready agg work behind a DMA
                    # wait.
                    if si + la_att < s:
                        att_part(si + la_att)
                    if si + la < s:
                        pooled_part(si + la)
                    convs(si)

            if loop_n > 1:
                with tc.For_i(0, loop_n, 1, hint_engines=(mybir.EngineType.PE,
                        mybir.EngineType.Activation, mybir.EngineType.DVE)):
                    body()
            else:
                body()
    nc.compile()
    return nc


def prep_inputs(x, w_attn1, w_attn2, weight, bias):
    """Host-side layout/dtype transforms (no math beyond scaling W1 by 1/L)."""
    x = np.asarray(x, dtype=np.float32)
    bs, c_in, length = x.shape
    lp = length + 2 * PAD
    xb = x.astype(ml_dtypes.bfloat16)
    xd = np.zeros((bs, 128, lp), dtype=ml_dtypes.bfloat16)
    xd[:, :c_in, PAD : PAD + length] = xb
    # rows 64..127: shifted left by one (xd_hi[c] = xp[c+1])
    xd[:, 64 : 64 + c_in, PAD - 1 : PAD - 1 + length] = xb

    # both partition halves of xd sum to the same pooled total, and the
    # attention matmul contracts over all 128 partitions -> divide by 2
    w1t = (np.asarray(w_attn1, np.float32) / float(length)).T  # [C_in, H]
    w1d = np.ascontiguousarray(np.vstack([w1t, w1t]))  # [128, H]
    w2t = np.asarray(w_attn2, np.float32).T.copy()  # [H, K]

    w = np.asarray(weight, np.float32)  # [K, C_out, C_in, KS]
    wbk = np.zeros((K, 128, 512), dtype=np.float32)
    for f in range(KS):
        half, pair = f % 2, f // 2
        wbk[:, half * 64 : half * 64 + c_in, pair * 128 : pair * 128 + C_OUT] = (
            w[:, :, :, f].transpose(0, 2, 1)
        )
    bkbt = np.asarray(bias, np.float32).T.copy()  # [C_out, K]
    return xd, w1d, w2t, wbk.astype(ml_dtypes.bfloat16), bkbt


def kernel(x, w_attn1, w_attn2, weight, bias):
    xd, w1d, w2t, wbk, bkbt = prep_inputs(x, w_attn1, w_attn2, weight, bias)

    if "nc" not in _NC_CACHE:
        _NC_CACHE["nc"] = build_nc()
    nc = _NC_CACHE["nc"]

    in_maps = []
    for c in range(N_CORES):
        in_maps.append(
            {
                "xd": np.ascontiguousarray(xd[c * S : (c + 1) * S]),
                "w1d": w1d,
                "w2t": w2t,
                "wbk": wbk,
                "bkbt": bkbt,
            }
        )
    res = run_bass_kernel_spmd(nc, in_maps, core_ids=list(range(N_CORES)))
    outs = [res.results[c]["out"] for c in range(N_CORES)]
    return np.concatenate(outs, axis=0).astype(np.float32)
